# revision 1
# baseline (speedup 1.0000x reference)
"""Trainium2 Bass kernel for nn_DKEncoder (scatter_memory).

Math (per batch b, reformulated from the reference):
  qiL  = tanh(q0 @ WqL.T + bqL)                 (L in {2,1}, tiny)
  qpL  = qiL @ (WkvL / sqrt(100))               (fold the 1/sqrt(kd) scale)
  att2 = k2.flat(6144,100) @ qp2                (PE fp32r, k2 host-transposed)
  a2   = masked-softmax_d(leaky_relu(att2))     (partition-group softmax)
  c2   = sum_d a2 * v2                          (PE fp32r, block-diag selector)
  att1 = k1.flat(384,100) @ qp1
  a1   = masked-softmax_c(leaky_relu(att1))
  out  = sum_c a1 * concat([v1, c2], -1)        (PE fp32, accumulated selector)
  scatter rows to nonzero input_ent positions   (PE fp32, 0/1 gather matmul)

Sharding: pure data parallel, 4 batches per core across 8 cores.
All input-dependent data flows through DRAM parameters, so the program
is compiled once and reused for any inputs.

Layout notes:
- small constants are packed into one [128, CPACK] DMA
- attention runs in two batch-halves so c2t/out1 for half 0 overlap the
  (DMA-gated) attention matmuls of half 1
- fp32r matmuls need N>=2, so qp carries a zero pad column and the
  attention PSUM keeps [real, garbage] column pairs
"""

import math
from contextlib import ExitStack

import numpy as np

import concourse.bacc as bacc
import concourse.bass as bass
import concourse.mybir as mybir
import concourse.tile as tile

B, S, E, C, D, KD, QD = 32, 128, 24, 16, 16, 100, 768
NCORES = 8
BPC = B // NCORES          # batches per core
EC = E * C                 # 384 (e,c) rows
ROWS2 = EC * D             # 6144 (e,c,d) rows
NT2 = ROWS2 // 128         # 48 layer-0 tiles per batch
NT1 = EC // 128            # 3 layer-1 tiles per batch
NQ = QD // 128             # 6 q-chunks
OD = 2 * KD                # 200 output dim
F32 = mybir.dt.float32
F32R = mybir.dt.float32r
AF = mybir.ActivationFunctionType
OP = mybir.AluOpType
USE_F32R = True  # single-pass PE matmuls for the big streaming contractions
FB = F32R if USE_F32R else F32

# packed-constants layout: name -> (rows, width)
CPACK_FIELDS = [
    ("q0t", 128, NQ * BPC),
    ("wq2t", 128, NQ * KD),
    ("wq1t", 128, NQ * KD),
    ("m24", 128, NT1 * E),
    ("sel16", 128, 8),
    ("wkv2", KD, KD),
    ("wkv1", KD, KD),
    ("bq2", KD, 1),
    ("bq1", KD, 1),
    ("ident", KD, KD),
    ("rep16", 8, 128),
    ("gmat", E, BPC * 128),
]
CPACK_W = sum(w for _, _, w in CPACK_FIELDS)
CPACK_OFF = {}
_off = 0
for _n, _r, _w in CPACK_FIELDS:
    CPACK_OFF[_n] = _off
    _off += _w


def build_nc() -> bass.Bass:
    nc = bacc.Bacc(None)
    p = lambda name, shape, out=False, dt=F32: nc.declare_dram_parameter(
        name, list(shape), dt, isOutput=out)

    k2t = p("k2t", [BPC, KD, ROWS2], dt=FB)  # per batch: k2 flat transposed
    v2r = p("v2r", [BPC, 128, NT2 * KD], dt=FB)  # per batch: v2 rows tiled
    k1t = p("k1t", [KD, BPC * EC], dt=FB)    # k1 flat transposed
    v1r = p("v1r", [128, BPC * NT1 * KD])    # v1 rows tiled
    cpack = p("cpack", [128, CPACK_W])       # all small constants
    out = p("out", [BPC, 128, OD], out=True)

    with tile.TileContext(nc) as tc, ExitStack() as ctx:
        _body(ctx, tc, nc, locals())
    nc.compile()
    return nc


def _body(ctx, tc, nc, t):
    consts = ctx.enter_context(tc.tile_pool(name="consts", bufs=1))

    cp = consts.tile([128, CPACK_W], F32, tag="cpack")
    nc.sync.dma_start(cp[:], t["cpack"][:])

    def cc(name):
        rows, w = next((r, w) for n, r, w in CPACK_FIELDS if n == name)
        o = CPACK_OFF[name]
        return cp[0:rows, o:o + w]

    q0t, wq2t, wq1t, m24, sel16 = cc("q0t"), cc("wq2t"), cc("wq1t"), cc("m24"), cc("sel16")
    wkv2, wkv1, bq2, bq1 = cc("wkv2"), cc("wkv1"), cc("bq2"), cc("bq1")
    ident, rep16, gmat = cc("ident"), cc("rep16"), cc("gmat")

    k1t = consts.tile([KD, BPC * EC], FB, tag="k1t")
    nc.sync.dma_start(k1t[:], t["k1t"][:])
    v1r = consts.tile([128, BPC * NT1 * KD], F32, tag="v1r")
    nc.sync.dma_start(v1r[:], t["v1r"][:])

    work = ctx.enter_context(tc.tile_pool(name="work", bufs=1))
    k2pool = ctx.enter_context(tc.tile_pool(name="k2t", bufs=3))
    v2pool = ctx.enter_context(tc.tile_pool(name="v2r", bufs=3))

    # ---- Phase Q: qp2/qp1 [100, BPC+1] (zero pad col for fp32r N=2) ----
    qp = {}
    with tc.tile_pool(name="ps_q", bufs=2, space="PSUM") as ps_q:
        for lname, wqt, wkv, bq in (("qp2", wq2t, wkv2, bq2), ("qp1", wq1t, wkv1, bq1)):
            qtmp = ps_q.tile([KD, BPC], F32, tag="qtmp")
            for c in range(NQ):
                nc.tensor.matmul(
                    qtmp[:],
                    wqt[:, c * KD:(c + 1) * KD],
                    q0t[:, c * BPC:(c + 1) * BPC],
                    start=(c == 0), stop=(c == NQ - 1),
                )
            qi = work.tile([KD, BPC], F32, tag="qi")
            nc.scalar.activation(qi[:], qtmp[:], AF.Tanh, bias=bq[:, 0:1], scale=1.0)
            qps = ps_q.tile([KD, BPC], F32, tag="qps")
            nc.tensor.matmul(qps[:], wkv[:], qi[:], start=True, stop=True)
            qsb = work.tile([KD, BPC + 1], FB, tag=lname)
            nc.vector.tensor_copy(qsb[:, 0:BPC], qps[:])
            nc.vector.memset(qsb[:, BPC:BPC + 1].bitcast(F32), 0.0)
            qp[lname] = qsb

    att_sel = work.tile([128, BPC * NT2 * 8], FB, tag="att_sel")
    sel24 = work.tile([128, BPC * NT1 * E], F32, tag="sel24")

    ps_att = ctx.enter_context(tc.tile_pool(name="ps_att", bufs=1, space="PSUM"))
    ps_sm = ctx.enter_context(tc.tile_pool(name="ps_sm", bufs=1, space="PSUM"))
    ps_c2 = ctx.enter_context(tc.tile_pool(name="ps_c2", bufs=2, space="PSUM"))
    ps_tp = ctx.enter_context(tc.tile_pool(name="ps_tp", bufs=1, space="PSUM"))
    ps_o1 = ctx.enter_context(tc.tile_pool(name="ps_o1", bufs=1, space="PSUM"))
    ps_g = ctx.enter_context(tc.tile_pool(name="ps_g", bufs=1, space="PSUM"))

    # group-of-16 partition softmax over a [128, nc2] range holding
    # [real, garbage] column pairs in PSUM; returns dense [128, ncols] SBUF
    def softmax(att_pair_view, ncols, tg):
        att_sb = work.tile([128, ncols], F32, tag=tg + "att")
        nc.scalar.activation(att_sb[:].unsqueeze(2), att_pair_view, AF.Copy)
        mask = work.tile([128, ncols], F32, tag=tg + "mask")
        nc.vector.tensor_scalar(mask[:], att_sb[:], 0.0, None, op0=OP.not_equal)
        lr = work.tile([128, ncols], F32, tag=tg + "lr")
        nc.vector.scalar_tensor_tensor(
            lr[:], att_sb[:], 0.01, att_sb[:], op0=OP.mult, op1=OP.max)
        ex = work.tile([128, ncols], F32, tag=tg + "ex")
        nc.scalar.activation(ex[:], lr[:], AF.Exp)
        exm = work.tile([128, ncols], F32, tag=tg + "exm")
        nc.vector.tensor_mul(exm[:], ex[:], mask[:])
        sums_ps = ps_sm.tile([8, ncols], F32, tag="sm_ps")
        nc.tensor.matmul(sums_ps[:], sel16[:], exm[:], start=True, stop=True)
        sums = work.tile([8, ncols], F32, tag=tg + "sumsb")
        nc.vector.tensor_scalar_add(sums[:], sums_ps[:], 1e-30)
        lns = work.tile([8, ncols], F32, tag=tg + "ln")
        nc.scalar.activation(lns[:], sums[:], AF.Ln)
        rinv = work.tile([8, ncols], F32, tag=tg + "rinv")
        nc.scalar.activation(rinv[:], lns[:], AF.Exp, scale=-1.0)
        rrep_ps = ps_sm.tile([128, ncols], F32, tag="sm_ps")
        nc.tensor.matmul(rrep_ps[:], rep16[:], rinv[:], start=True, stop=True)
        attn = work.tile([128, ncols], F32, tag=tg + "attn")
        nc.vector.tensor_mul(attn[:], exm[:], rrep_ps[:])
        m2 = work.tile([128, ncols], F32, tag=tg + "m2")
        nc.vector.tensor_scalar(m2[:], attn[:], 1.0 / 16.0, None, op0=OP.not_equal)
        attf = work.tile([128, ncols], F32, tag=tg + "attf")
        nc.vector.tensor_mul(attf[:], attn[:], m2[:])
        return attf

    att2_ps = ps_att.tile([128, 2 * BPC * NT2], F32, tag="att2")
    att1_ps = ps_att.tile([128, 2 * BPC * NT1], F32, tag="att1")

    HALF = BPC // 2
    for h in range(2):
        js = range(h * HALF, (h + 1) * HALF)
        # ---- attention logits for this half ----
        for j in js:
            k2tile = k2pool.tile([KD, ROWS2], FB, tag="k2tile")
            nc.sync.dma_start(k2tile[:], t["k2t"][j, :, :])
            for tt in range(NT2):
                col = 2 * (j * NT2 + tt)
                nc.tensor.matmul(
                    att2_ps[:, col:col + 2],
                    k2tile[:, tt * 128:(tt + 1) * 128],
                    qp["qp2"][:, j:j + 2],
                    start=True, stop=True,
                )
            for tt in range(NT1):
                col = 2 * (j * NT1 + tt)
                nc.tensor.matmul(
                    att1_ps[:, col:col + 2],
                    k1t[:, j * EC + tt * 128: j * EC + (tt + 1) * 128],
                    qp["qp1"][:, j:j + 2],
                    start=True, stop=True,
                )

        # ---- softmax for this half ----
        n2, n1 = HALF * NT2, HALF * NT1
        a2view = att2_ps[:].rearrange("p (c two) -> p c two", two=2)[
            :, h * n2:(h + 1) * n2, 0:1]
        a1view = att1_ps[:].rearrange("p (c two) -> p c two", two=2)[
            :, h * n1:(h + 1) * n1, 0:1]
        att2f = softmax(a2view, n2, "s2_")
        att1f = softmax(a1view, n1, "s1_")

        # selector builds (0-step broadcast dims; mask picks the diagonal)
        nc.vector.tensor_mul(
            att_sel[:, h * n2 * 8:(h + 1) * n2 * 8].rearrange(
                "p (c g) -> p c g", g=8),
            att2f[:].unsqueeze(2).broadcast_to([128, n2, 8]),
            sel16[:].unsqueeze(1).broadcast_to([128, n2, 8]),
        )
        nc.vector.tensor_mul(
            sel24[:, h * n1 * E:(h + 1) * n1 * E].rearrange(
                "p (j t e) -> p j t e", j=HALF, t=NT1),
            att1f[:].rearrange("p (j t) -> p j t", j=HALF).unsqueeze(3)
            .broadcast_to([128, HALF, NT1, E]),
            m24[:].rearrange("p (t e) -> p t e", t=NT1).unsqueeze(1)
            .broadcast_to([128, HALF, NT1, E]),
        )

        # ---- combined2 (transposed), layer 1, gather, store ----
        for j in js:
            v2tile = v2pool.tile([128, NT2 * KD], FB, tag="v2tile")
            nc.sync.dma_start(v2tile[:], t["v2r"][j, :, :])
            c2t_ps = ps_c2.tile([KD, EC], F32, tag="c2t")
            for tt in range(NT2):
                nc.tensor.matmul(
                    c2t_ps[:, tt * 8:(tt + 1) * 8],
                    v2tile[:, tt * KD:(tt + 1) * KD],
                    att_sel[:, (j * NT2 + tt) * 8:(j * NT2 + tt + 1) * 8],
                    start=True, stop=True,
                )
            c2t = work.tile([KD, EC], F32, tag="c2t_sb")
            nc.vector.tensor_copy(c2t[:], c2t_ps[:])

            vcat = work.tile([128, NT1 * OD], F32, tag="vcat")
            for tt in range(NT1):
                nc.vector.tensor_copy(
                    vcat[:, tt * OD: tt * OD + KD],
                    v1r[:, (j * NT1 + tt) * KD:(j * NT1 + tt + 1) * KD],
                )
                tp_ps = ps_tp.tile([128, KD], F32, tag="tp")
                nc.tensor.transpose(tp_ps[:], c2t[:, tt * 128:(tt + 1) * 128], ident[:])
                nc.vector.tensor_copy(vcat[:, tt * OD + KD:(tt + 1) * OD], tp_ps[:])

            out1_ps = ps_o1.tile([E, OD], F32, tag="out1")
            for tt in range(NT1):
                nc.tensor.matmul(
                    out1_ps[:],
                    sel24[:, (j * NT1 + tt) * E:(j * NT1 + tt + 1) * E],
                    vcat[:, tt * OD:(tt + 1) * OD],
                    start=(tt == 0), stop=(tt == NT1 - 1),
                )
            table = work.tile([E, OD], F32, tag="table")
            nc.vector.tensor_copy(table[:], out1_ps[:])

            g_ps = ps_g.tile([128, OD], F32, tag="gath")
            nc.tensor.matmul(
                g_ps[:], gmat[:, j * 128:(j + 1) * 128], table[:],
                start=True, stop=True,
            )
            osb = work.tile([128, OD], F32, tag="osb")
            nc.vector.tensor_copy(osb[:], g_ps[:])
            nc.sync.dma_start(t["out"][j, :, :], osb[:])


def prep_inputs(inputs: dict) -> list[dict]:
    """Split full inputs into per-core input maps (host-side relayout only)."""
    q = np.ascontiguousarray(inputs["q"][:, 0, :], dtype=np.float32)      # [B, 768]
    k1 = np.asarray(inputs["k1"], dtype=np.float32)
    v1 = np.asarray(inputs["v1"], dtype=np.float32)
    k2 = np.asarray(inputs["k2"], dtype=np.float32)
    v2 = np.asarray(inputs["v2"], dtype=np.float32)
    ent = np.asarray(inputs["input_ent"])

    scale = np.float32(1.0 / math.sqrt(KD))
    wkv2 = np.asarray(inputs["Wkv2"], np.float32) * scale
    wkv1 = np.asarray(inputs["Wkv1"], np.float32) * scale
    wq2t = (np.asarray(inputs["Wq2"], np.float32).T.reshape(NQ, 128, KD)
            .transpose(1, 0, 2).reshape(128, NQ * KD))
    wq1t = (np.asarray(inputs["Wq1"], np.float32).T.reshape(NQ, 128, KD)
            .transpose(1, 0, 2).reshape(128, NQ * KD))
    bq2 = np.asarray(inputs["bq2"], np.float32).reshape(KD, 1)
    bq1 = np.asarray(inputs["bq1"], np.float32).reshape(KD, 1)

    pp = np.arange(128)
    sel16 = (pp[:, None] // 16 == np.arange(8)[None, :]).astype(np.float32)
    rep16 = np.ascontiguousarray(sel16.T)
    te = np.arange(NT1 * E)
    m24 = (te[None, :] % E == 8 * (te[None, :] // E) + pp[:, None] // 16).astype(np.float32)
    ident = np.eye(KD, dtype=np.float32)

    mask = ent != 0
    rank = np.cumsum(mask, axis=1) - 1

    base = {"q0t": None, "wq2t": wq2t, "wq1t": wq1t, "m24": m24,
            "sel16": sel16, "wkv2": wkv2, "wkv1": wkv1, "bq2": bq2,
            "bq1": bq1, "ident": ident, "rep16": rep16, "gmat": None}

    maps = []
    for i in range(NCORES):
        bs = slice(i * BPC, (i + 1) * BPC)
        k2c, v2c = k2[bs], v2[bs]
        k1c, v1c = k1[bs], v1[bs]
        k2tc = np.ascontiguousarray(
            k2c.reshape(BPC, ROWS2, KD).transpose(0, 2, 1))             # [4,100,6144]
        v2rc = np.ascontiguousarray(
            v2c.reshape(BPC, NT2, 128, KD).transpose(0, 2, 1, 3)
            .reshape(BPC, 128, NT2 * KD))                                # [4,128,4800]
        k1tc = np.ascontiguousarray(
            k1c.reshape(BPC, EC, KD).transpose(2, 0, 1).reshape(KD, BPC * EC))
        v1rc = np.ascontiguousarray(
            v1c.reshape(BPC, NT1, 128, KD).transpose(2, 0, 1, 3)
            .reshape(128, BPC * NT1 * KD))
        q0tc = (q[bs].T.reshape(NQ, 128, BPC).transpose(1, 0, 2)
                .reshape(128, NQ * BPC))
        gm = np.zeros((E, BPC * 128), np.float32)
        for j in range(BPC):
            b = i * BPC + j
            for s in range(S):
                if mask[b, s]:
                    gm[rank[b, s], j * 128 + s] = 1.0

        cpk = np.zeros((128, CPACK_W), np.float32)
        vals = dict(base)
        vals["q0t"] = q0tc
        vals["gmat"] = gm
        for name, rows, w in CPACK_FIELDS:
            o = CPACK_OFF[name]
            cpk[0:rows, o:o + w] = vals[name]

        maps.append({
            "k2t": k2tc, "v2r": v2rc, "k1t": k1tc, "v1r": v1rc,
            "cpack": cpk,
        })
    return maps


_NC_CACHE = {}


def kernel(**inputs) -> np.ndarray:
    from concourse.bass_utils import run_bass_kernel_spmd

    if "nc" not in _NC_CACHE:
        _NC_CACHE["nc"] = build_nc()
    nc = _NC_CACHE["nc"]
    maps = prep_inputs(inputs)
    res = run_bass_kernel_spmd(nc, maps, list(range(NCORES))).results
    out = np.concatenate([res[i]["out"] for i in range(NCORES)], axis=0)
    return np.ascontiguousarray(out.reshape(B, S, OD).astype(np.float32))



# revision 7
# speedup vs baseline: 1.8341x; 1.8341x over previous
"""Trainium2 Bass kernel for nn_DKEncoder (scatter_memory).

Math (per batch b, reformulated from the reference):
  qiL  = tanh(q0 @ WqL.T + bqL)                 (L in {2,1}, tiny)
  qpL  = qiL @ (WkvL / sqrt(100))               (fold the 1/sqrt(kd) scale)
  att2 = k2.flat(6144,100) @ qp2                (PE fp32r, k2 host-transposed)
  a2   = masked-softmax_d(leaky_relu(att2))     (partition-group softmax)
  c2   = sum_d a2 * v2                          (PE fp32r, block-diag selector)
  att1 = k1.flat(384,100) @ qp1
  a1   = masked-softmax_c(leaky_relu(att1))
  out  = sum_c a1 * concat([v1, c2], -1)        (PE fp32, accumulated selector)
  scatter rows to nonzero input_ent positions   (PE fp32, 0/1 gather matmul)

Sharding: pure data parallel, 4 batches per core across 8 cores.
All input-dependent data flows through DRAM parameters, so the program
is compiled once and reused for any inputs.

Layout notes:
- small constants are packed into one [128, CPACK] DMA
- attention runs in two batch-halves so c2t/out1 for half 0 overlap the
  (DMA-gated) attention matmuls of half 1
- fp32r matmuls need N>=2, so qp carries a zero pad column and the
  attention PSUM keeps [real, garbage] column pairs
"""

import math
from contextlib import ExitStack

import ml_dtypes
import numpy as np

BF16NP = ml_dtypes.bfloat16

import concourse.bacc as bacc
import concourse.bass as bass
import concourse.mybir as mybir
import concourse.tile as tile

B, S, E, C, D, KD, QD = 32, 128, 24, 16, 16, 100, 768
NCORES = 8
BPC = B // NCORES          # batches per core
EC = E * C                 # 384 (e,c) rows
ROWS2 = EC * D             # 6144 (e,c,d) rows
NT2 = ROWS2 // 128         # 48 layer-0 tiles per batch
NT1 = EC // 128            # 3 layer-1 tiles per batch
NQ = QD // 128             # 6 q-chunks
OD = 2 * KD                # 200 output dim
F32 = mybir.dt.float32
F32R = mybir.dt.float32r
BF16 = mybir.dt.bfloat16
AF = mybir.ActivationFunctionType
OP = mybir.AluOpType
FB = BF16  # big streaming tensors in bf16: halves DMA bytes + PE loads

# packed-constants layout: name -> (rows, width)
CPACK_FIELDS = [
    ("q0t", 128, NQ * BPC),
    ("wq2t", 128, NQ * KD),
    ("wq1t", 128, NQ * KD),
    ("m24", 128, NT1 * E),
    ("sel16", 128, 8),
    ("wkv2", KD, KD),
    ("wkv1", KD, KD),
    ("bq2", KD, 1),
    ("bq1", KD, 1),
    ("ident", KD, KD),
    ("rep16", 8, 128),
    ("gmat", E, BPC * 128),
]
CPACK_W = sum(w for _, _, w in CPACK_FIELDS)
CPACK_OFF = {}
_off = 0
for _n, _r, _w in CPACK_FIELDS:
    CPACK_OFF[_n] = _off
    _off += _w


def build_nc() -> bass.Bass:
    nc = bacc.Bacc(None)
    p = lambda name, shape, out=False, dt=F32: nc.declare_dram_parameter(
        name, list(shape), dt, isOutput=out)

    k2t = p("k2t", [BPC, KD, ROWS2], dt=FB)  # per batch: k2 flat transposed
    v2r = p("v2r", [BPC, 128, NT2 * KD], dt=FB)  # per batch: v2 rows tiled
    k1t = p("k1t", [KD, BPC * EC], dt=FB)    # k1 flat transposed
    v1r = p("v1r", [128, BPC * NT1 * KD], dt=FB)  # v1 rows tiled
    cpack = p("cpack", [128, CPACK_W])       # all small constants
    out = p("out", [BPC, 128, OD], out=True)

    with tile.TileContext(nc) as tc, ExitStack() as ctx:
        _body(ctx, tc, nc, locals())
    nc.compile()
    return nc


def _body(ctx, tc, nc, t):
    consts = ctx.enter_context(tc.tile_pool(name="consts", bufs=1))

    cp = consts.tile([128, CPACK_W], F32, tag="cpack")
    nc.sync.dma_start(cp[:], t["cpack"][:])

    def cc(name):
        rows, w = next((r, w) for n, r, w in CPACK_FIELDS if n == name)
        o = CPACK_OFF[name]
        return cp[0:rows, o:o + w]

    q0t, wq2t, wq1t, m24, sel16 = cc("q0t"), cc("wq2t"), cc("wq1t"), cc("m24"), cc("sel16")
    wkv2, wkv1, bq2, bq1 = cc("wkv2"), cc("wkv1"), cc("bq2"), cc("bq1")
    ident, rep16, gmat = cc("ident"), cc("rep16"), cc("gmat")

    k1t = consts.tile([KD, BPC * EC], FB, tag="k1t")
    nc.sync.dma_start(k1t[:], t["k1t"][:])
    v1r = consts.tile([128, BPC * NT1 * KD], FB, tag="v1r")
    nc.sync.dma_start(v1r[:], t["v1r"][:])

    work = ctx.enter_context(tc.tile_pool(name="work", bufs=1))
    k2pool = ctx.enter_context(tc.tile_pool(name="k2t", bufs=3))
    v2pool = ctx.enter_context(tc.tile_pool(name="v2r", bufs=3))

    # ---- Phase Q: qp2/qp1 [100, BPC+1] (zero pad col for fp32r N=2) ----
    qp = {}
    with tc.tile_pool(name="ps_q", bufs=2, space="PSUM") as ps_q:
        for lname, wqt, wkv, bq in (("qp2", wq2t, wkv2, bq2), ("qp1", wq1t, wkv1, bq1)):
            qtmp = ps_q.tile([KD, BPC], F32, tag="qtmp")
            for c in range(NQ):
                nc.tensor.matmul(
                    qtmp[:],
                    wqt[:, c * KD:(c + 1) * KD],
                    q0t[:, c * BPC:(c + 1) * BPC],
                    start=(c == 0), stop=(c == NQ - 1),
                )
            qi = work.tile([KD, BPC], F32, tag="qi")
            nc.scalar.activation(qi[:], qtmp[:], AF.Tanh, bias=bq[:, 0:1], scale=1.0)
            qps = ps_q.tile([KD, BPC], F32, tag="qps")
            nc.tensor.matmul(qps[:], wkv[:], qi[:], start=True, stop=True)
            qsb = work.tile([KD, BPC + 1], FB, tag=lname)
            nc.vector.tensor_copy(qsb[:, 0:BPC], qps[:])
            nc.vector.memset(qsb[:, BPC:BPC + 1], 0.0)
            qp[lname] = qsb

    att_sel = work.tile([128, BPC * NT2 * 8], FB, tag="att_sel")
    sel24 = work.tile([128, BPC * NT1 * E], F32, tag="sel24")

    ps_att = ctx.enter_context(tc.tile_pool(name="ps_att", bufs=1, space="PSUM"))
    ps_sm = ctx.enter_context(tc.tile_pool(name="ps_sm", bufs=1, space="PSUM"))
    ps_c2 = ctx.enter_context(tc.tile_pool(name="ps_c2", bufs=2, space="PSUM"))
    ps_tp = ctx.enter_context(tc.tile_pool(name="ps_tp", bufs=1, space="PSUM"))
    ps_o1 = ctx.enter_context(tc.tile_pool(name="ps_o1", bufs=1, space="PSUM"))
    ps_g = ctx.enter_context(tc.tile_pool(name="ps_g", bufs=1, space="PSUM"))

    # group-of-16 partition softmax over a [128, nc2] range holding
    # [real, garbage] column pairs in PSUM; returns dense [128, ncols] SBUF
    def softmax(att_pair_view, ncols, tg):
        att_sb = work.tile([128, ncols], F32, tag=tg + "att")
        nc.scalar.activation(att_sb[:].unsqueeze(2), att_pair_view, AF.Copy)
        mask = work.tile([128, ncols], F32, tag=tg + "mask")
        nc.vector.tensor_scalar(mask[:], att_sb[:], 0.0, None, op0=OP.not_equal)
        lr = work.tile([128, ncols], F32, tag=tg + "lr")
        nc.vector.scalar_tensor_tensor(
            lr[:], att_sb[:], 0.01, att_sb[:], op0=OP.mult, op1=OP.max)
        ex = work.tile([128, ncols], F32, tag=tg + "ex")
        nc.scalar.activation(ex[:], lr[:], AF.Exp)
        exm = work.tile([128, ncols], F32, tag=tg + "exm")
        nc.vector.tensor_mul(exm[:], ex[:], mask[:])
        sums_ps = ps_sm.tile([8, ncols], F32, tag="sm_ps")
        nc.tensor.matmul(sums_ps[:], sel16[:], exm[:], start=True, stop=True)
        sums = work.tile([8, ncols], F32, tag=tg + "sumsb")
        nc.vector.tensor_scalar_add(sums[:], sums_ps[:], 1e-30)
        lns = work.tile([8, ncols], F32, tag=tg + "ln")
        nc.scalar.activation(lns[:], sums[:], AF.Ln)
        rinv = work.tile([8, ncols], F32, tag=tg + "rinv")
        nc.scalar.activation(rinv[:], lns[:], AF.Exp, scale=-1.0)
        rrep_ps = ps_sm.tile([128, ncols], F32, tag="sm_ps")
        nc.tensor.matmul(rrep_ps[:], rep16[:], rinv[:], start=True, stop=True)
        attn = work.tile([128, ncols], F32, tag=tg + "attn")
        nc.vector.tensor_mul(attn[:], exm[:], rrep_ps[:])
        m2 = work.tile([128, ncols], F32, tag=tg + "m2")
        nc.vector.tensor_scalar(m2[:], attn[:], 1.0 / 16.0, None, op0=OP.not_equal)
        attf = work.tile([128, ncols], F32, tag=tg + "attf")
        nc.vector.tensor_mul(attf[:], attn[:], m2[:])
        return attf

    att2_ps = ps_att.tile([128, 2 * BPC * NT2], F32, tag="att2")
    att1_ps = ps_att.tile([128, 2 * BPC * NT1], F32, tag="att1")

    HALF = BPC // 2
    for h in range(2):
        js = range(h * HALF, (h + 1) * HALF)
        # ---- attention logits for this half ----
        for j in js:
            k2tile = k2pool.tile([KD, ROWS2], FB, tag="k2tile")
            nc.sync.dma_start(k2tile[:], t["k2t"][j, :, :])
            for tt in range(NT2):
                col = 2 * (j * NT2 + tt)
                nc.tensor.matmul(
                    att2_ps[:, col:col + 2],
                    k2tile[:, tt * 128:(tt + 1) * 128],
                    qp["qp2"][:, j:j + 2],
                    start=True, stop=True,
                )
            for tt in range(NT1):
                col = 2 * (j * NT1 + tt)
                nc.tensor.matmul(
                    att1_ps[:, col:col + 2],
                    k1t[:, j * EC + tt * 128: j * EC + (tt + 1) * 128],
                    qp["qp1"][:, j:j + 2],
                    start=True, stop=True,
                )

        # ---- softmax for this half ----
        n2, n1 = HALF * NT2, HALF * NT1
        a2view = att2_ps[:].rearrange("p (c two) -> p c two", two=2)[
            :, h * n2:(h + 1) * n2, 0:1]
        a1view = att1_ps[:].rearrange("p (c two) -> p c two", two=2)[
            :, h * n1:(h + 1) * n1, 0:1]
        att2f = softmax(a2view, n2, "s2_")
        att1f = softmax(a1view, n1, "s1_")

        # selector builds (0-step broadcast dims; mask picks the diagonal)
        nc.vector.tensor_mul(
            att_sel[:, h * n2 * 8:(h + 1) * n2 * 8].rearrange(
                "p (c g) -> p c g", g=8),
            att2f[:].unsqueeze(2).broadcast_to([128, n2, 8]),
            sel16[:].unsqueeze(1).broadcast_to([128, n2, 8]),
        )
        nc.vector.tensor_mul(
            sel24[:, h * n1 * E:(h + 1) * n1 * E].rearrange(
                "p (j t e) -> p j t e", j=HALF, t=NT1),
            att1f[:].rearrange("p (j t) -> p j t", j=HALF).unsqueeze(3)
            .broadcast_to([128, HALF, NT1, E]),
            m24[:].rearrange("p (t e) -> p t e", t=NT1).unsqueeze(1)
            .broadcast_to([128, HALF, NT1, E]),
        )

        # ---- combined2 (transposed), layer 1, gather, store ----
        for j in js:
            v2tile = v2pool.tile([128, NT2 * KD], FB, tag="v2tile")
            nc.sync.dma_start(v2tile[:], t["v2r"][j, :, :])
            c2t_ps = ps_c2.tile([KD, EC], F32, tag="c2t")
            for tt in range(NT2):
                nc.tensor.matmul(
                    c2t_ps[:, tt * 8:(tt + 1) * 8],
                    v2tile[:, tt * KD:(tt + 1) * KD],
                    att_sel[:, (j * NT2 + tt) * 8:(j * NT2 + tt + 1) * 8],
                    start=True, stop=True,
                )
            c2t = work.tile([KD, EC], F32, tag="c2t_sb")
            nc.vector.tensor_copy(c2t[:], c2t_ps[:])

            vcat = work.tile([128, NT1 * OD], F32, tag="vcat")
            for tt in range(NT1):
                nc.vector.tensor_copy(
                    vcat[:, tt * OD: tt * OD + KD],
                    v1r[:, (j * NT1 + tt) * KD:(j * NT1 + tt + 1) * KD],
                )
                tp_ps = ps_tp.tile([128, KD], F32, tag="tp")
                nc.tensor.transpose(tp_ps[:], c2t[:, tt * 128:(tt + 1) * 128], ident[:])
                nc.vector.tensor_copy(vcat[:, tt * OD + KD:(tt + 1) * OD], tp_ps[:])

            out1_ps = ps_o1.tile([E, OD], F32, tag="out1")
            for tt in range(NT1):
                nc.tensor.matmul(
                    out1_ps[:],
                    sel24[:, (j * NT1 + tt) * E:(j * NT1 + tt + 1) * E],
                    vcat[:, tt * OD:(tt + 1) * OD],
                    start=(tt == 0), stop=(tt == NT1 - 1),
                )
            table = work.tile([E, OD], F32, tag="table")
            nc.vector.tensor_copy(table[:], out1_ps[:])

            g_ps = ps_g.tile([128, OD], F32, tag="gath")
            nc.tensor.matmul(
                g_ps[:], gmat[:, j * 128:(j + 1) * 128], table[:],
                start=True, stop=True,
            )
            osb = work.tile([128, OD], F32, tag="osb")
            nc.vector.tensor_copy(osb[:], g_ps[:])
            nc.sync.dma_start(t["out"][j, :, :], osb[:])


def prep_inputs(inputs: dict) -> list[dict]:
    """Split full inputs into per-core input maps (host-side relayout only)."""
    q = np.ascontiguousarray(inputs["q"][:, 0, :], dtype=np.float32)      # [B, 768]
    k1 = np.asarray(inputs["k1"], dtype=np.float32)
    v1 = np.asarray(inputs["v1"], dtype=np.float32)
    k2 = np.asarray(inputs["k2"], dtype=np.float32)
    v2 = np.asarray(inputs["v2"], dtype=np.float32)
    ent = np.asarray(inputs["input_ent"])

    scale = np.float32(1.0 / math.sqrt(KD))
    wkv2 = np.asarray(inputs["Wkv2"], np.float32) * scale
    wkv1 = np.asarray(inputs["Wkv1"], np.float32) * scale
    wq2t = (np.asarray(inputs["Wq2"], np.float32).T.reshape(NQ, 128, KD)
            .transpose(1, 0, 2).reshape(128, NQ * KD))
    wq1t = (np.asarray(inputs["Wq1"], np.float32).T.reshape(NQ, 128, KD)
            .transpose(1, 0, 2).reshape(128, NQ * KD))
    bq2 = np.asarray(inputs["bq2"], np.float32).reshape(KD, 1)
    bq1 = np.asarray(inputs["bq1"], np.float32).reshape(KD, 1)

    pp = np.arange(128)
    sel16 = (pp[:, None] // 16 == np.arange(8)[None, :]).astype(np.float32)
    rep16 = np.ascontiguousarray(sel16.T)
    te = np.arange(NT1 * E)
    m24 = (te[None, :] % E == 8 * (te[None, :] // E) + pp[:, None] // 16).astype(np.float32)
    ident = np.eye(KD, dtype=np.float32)

    mask = ent != 0
    rank = np.cumsum(mask, axis=1) - 1

    base = {"q0t": None, "wq2t": wq2t, "wq1t": wq1t, "m24": m24,
            "sel16": sel16, "wkv2": wkv2, "wkv1": wkv1, "bq2": bq2,
            "bq1": bq1, "ident": ident, "rep16": rep16, "gmat": None}

    maps = []
    for i in range(NCORES):
        bs = slice(i * BPC, (i + 1) * BPC)
        k2c, v2c = k2[bs], v2[bs]
        k1c, v1c = k1[bs], v1[bs]
        k2tc = np.ascontiguousarray(
            k2c.reshape(BPC, ROWS2, KD).transpose(0, 2, 1)).astype(BF16NP)
        v2rc = np.ascontiguousarray(
            v2c.reshape(BPC, NT2, 128, KD).transpose(0, 2, 1, 3)
            .reshape(BPC, 128, NT2 * KD)).astype(BF16NP)
        k1tc = np.ascontiguousarray(
            k1c.reshape(BPC, EC, KD).transpose(2, 0, 1)
            .reshape(KD, BPC * EC)).astype(BF16NP)
        v1rc = np.ascontiguousarray(
            v1c.reshape(BPC, NT1, 128, KD).transpose(2, 0, 1, 3)
            .reshape(128, BPC * NT1 * KD)).astype(BF16NP)
        q0tc = (q[bs].T.reshape(NQ, 128, BPC).transpose(1, 0, 2)
                .reshape(128, NQ * BPC))
        gm = np.zeros((E, BPC * 128), np.float32)
        for j in range(BPC):
            b = i * BPC + j
            for s in range(S):
                if mask[b, s]:
                    gm[rank[b, s], j * 128 + s] = 1.0

        cpk = np.zeros((128, CPACK_W), np.float32)
        vals = dict(base)
        vals["q0t"] = q0tc
        vals["gmat"] = gm
        for name, rows, w in CPACK_FIELDS:
            o = CPACK_OFF[name]
            cpk[0:rows, o:o + w] = vals[name]

        maps.append({
            "k2t": k2tc, "v2r": v2rc, "k1t": k1tc, "v1r": v1rc,
            "cpack": cpk,
        })
    return maps


_NC_CACHE = {}


def kernel(**inputs) -> np.ndarray:
    from concourse.bass_utils import run_bass_kernel_spmd

    if "nc" not in _NC_CACHE:
        _NC_CACHE["nc"] = build_nc()
    nc = _NC_CACHE["nc"]
    maps = prep_inputs(inputs)
    res = run_bass_kernel_spmd(nc, maps, list(range(NCORES))).results
    out = np.concatenate([res[i]["out"] for i in range(NCORES)], axis=0)
    return np.ascontiguousarray(out.reshape(B, S, OD).astype(np.float32))

